# revision 16
# baseline (speedup 1.0000x reference)
"""Trainium2 Bass kernel for nn_BaseEngine (8-core SPMD).

Computation (per reference):
  c = [x_bcast, h]  [N,192]
  out = (relu(c@W1a.T+b1a)@W2a.T+b2a) - (relu(c@W1g.T+b1g)@W2g.T+b2g)
  t = mean(out^2, -1)
  new_h = GRU([out, t], h)
  new_h = faction_sync(new_h, 8 factions, step)
  pred = softmax(t) @ out @ Wo.T + bo

Sharding: n_cells across 8 cores (16384 each); each faction == one core's
shard so faction means are core-local. One 1KB AllReduce carries
[faction_mean_row, weighted_out_row, sum_t, sum_exp_t].

Device layout: feature-major ([feature, cell] on [partition, free]) with
PE transposes at load/store. x is folded into layer-1 biases on the host,
b2 into the GRU input bias. Sigmoid is computed as 0.5+0.5*tanh(v/2) so
every ACT function stays in the one "exp_and_others" table set.
"""

import sys

sys.path.insert(0, "/opt/trn_rl_repo")

import numpy as np

import concourse.bass as bass
import concourse.mybir as mybir
from concourse import tile
from concourse.bass_utils import run_bass_kernel_spmd

N_CELLS = 131072
N_CORES = 8
N_LOCAL = N_CELLS // N_CORES  # 16384
CHUNK = 512
N_CHUNKS = N_LOCAL // CHUNK  # 32
GROUPS = CHUNK // 128  # 4
IN_D, HID_D, OUT_D = 64, 128, 64
SYNC, DEBATE = 0.15, 0.15
DC_CHUNKS = (N_LOCAL // 4) // CHUNK  # debate applies to first fs//4 cells

F32 = mybir.dt.float32
F32R = mybir.dt.float32r


def _r(ap):
    return ap.bitcast(F32R)
AF = mybir.ActivationFunctionType
ALU = mybir.AluOpType
AX = mybir.AxisListType

_CACHED = {}


def build_nc(do_debate: bool, repeat: int = 1) -> bass.Bass:
    nc = bass.Bass()

    h_in = nc.declare_dram_parameter("h", [N_LOCAL, HID_D], F32, isOutput=False)
    w1aT = nc.declare_dram_parameter("w1aT", [HID_D, 128], F32, isOutput=False)
    w1gT = nc.declare_dram_parameter("w1gT", [HID_D, 128], F32, isOutput=False)
    w2aT = nc.declare_dram_parameter("w2aT", [128, OUT_D], F32, isOutput=False)
    w2gTn = nc.declare_dram_parameter("w2gTn", [128, OUT_D], F32, isOutput=False)
    wihT = nc.declare_dram_parameter("wihT", [OUT_D + 1, 3 * HID_D], F32, isOutput=False)
    whhT = nc.declare_dram_parameter("whhT", [HID_D, 3 * HID_D], F32, isOutput=False)
    eye_in = nc.declare_dram_parameter("eye", [128, 128], F32, isOutput=False)
    ones_col_in = nc.declare_dram_parameter("ones_col", [64, 1], F32, isOutput=False)
    ones_row_in = nc.declare_dram_parameter("ones_row", [1, 64], F32, isOutput=False)
    bias_in = nc.declare_dram_parameter("biases", [128, 8], F32, isOutput=False)

    newh_out = nc.declare_dram_parameter("new_h", [N_LOCAL, HID_D], F32, isOutput=True)
    ar_param = nc.declare_dram_parameter("ar_out", [256], F32, isOutput=True)

    # [chunk, partition, (group, feat)] views of the cell-major DRAM tensors
    h_ap = h_in.ap().rearrange("(c g p) f -> c p g f", g=GROUPS, p=128)
    newh_ap = newh_out.ap().rearrange("(c g p) f -> c p g f", g=GROUPS, p=128)

    with tile.TileContext(nc) as tc:
        with (
            tc.tile_pool(name="const", bufs=1) as cpool,
            tc.tile_pool(name="acc", bufs=1) as apool,
            tc.tile_pool(name="big", bufs=1) as bigpool,
            tc.tile_pool(name="work", bufs=3) as wpool,
            tc.tile_pool(name="psum", bufs=2, space="PSUM") as ppool,
            tc.tile_pool(name="dram", bufs=1, space="DRAM") as dpool,
        ):
            # ---- constants / weights ----
            w1aT_sb = cpool.tile([HID_D, 128], F32, tag="w1a")
            w1gT_sb = cpool.tile([HID_D, 128], F32, tag="w1g")
            w2aT_sb = cpool.tile([128, OUT_D], F32, tag="w2a")
            w2gTn_sb = cpool.tile([128, OUT_D], F32, tag="w2g")
            wihT_sb = cpool.tile([OUT_D + 1, 3 * HID_D], F32, tag="wih")
            whhT_sb = cpool.tile([HID_D, 3 * HID_D], F32, tag="whh")
            eye = cpool.tile([128, 128], F32, tag="eye")
            bias_sb = cpool.tile([128, 8], F32, tag="bias")
            ones64div = cpool.tile([64, 1], F32, tag="o64")
            ones_row = cpool.tile([1, 64], F32, tag="orow")
            nc.sync.dma_start(_r(ones64div[:]), _r(ones_col_in.ap()))
            nc.sync.dma_start(_r(ones_row[:]), _r(ones_row_in.ap()))
            nc.sync.dma_start(_r(w1aT_sb[:]), _r(w1aT.ap()))
            nc.sync.dma_start(_r(w1gT_sb[:]), _r(w1gT.ap()))
            nc.sync.dma_start(_r(w2aT_sb[:]), _r(w2aT.ap()))
            nc.sync.dma_start(_r(w2gTn_sb[:]), _r(w2gTn.ap()))
            nc.sync.dma_start(_r(wihT_sb[:]), _r(wihT.ap()))
            nc.sync.dma_start(_r(whhT_sb[:]), _r(whhT.ap()))
            nc.sync.dma_start(_r(eye[:]), _r(eye_in.ap()))
            nc.sync.dma_start(_r(bias_sb[:]), _r(bias_in.ap()))

            # Pre-touch every PE-consumed tile with a tiny K=1 matmul so the
            # PE vector clock observes the load semaphores up front; real
            # matmuls then carry at most one sync wait (walrus's LDWEIGHTS
            # struct can't encode more).
            touch_ps = ppool.tile([1, 16], F32, tag="hT", bufs=1)
            for i, w in enumerate(
                [w1aT_sb, w1gT_sb, w2aT_sb, w2gTn_sb, wihT_sb, whhT_sb,
                 eye, ones64div, ones_row]
            ):
                nc.tensor.matmul(
                    touch_ps[0:1, i : i + 1], w[0:1, 0:1], w[0:1, 0:1],
                    start=True, stop=True,
                )

            b1a = bias_sb[:, 0:1]
            b1g = bias_sb[:, 1:2]
            br_half = bias_sb[:, 2:3]
            bz_half = bias_sb[:, 3:4]
            bih_n = bias_sb[:, 4:5]
            bhh_n = bias_sb[:, 5:6]
            b2eff = bias_sb[0:OUT_D, 6:7]

            for _rep in range(repeat):
                _emit_step(
                    nc, tc, cpool, apool, bigpool, wpool, ppool, dpool,
                    h_ap if _rep == 0 else newh_ap, newh_ap, ar_param,
                    do_debate,
                    dict(w1aT_sb=w1aT_sb, w1gT_sb=w1gT_sb, w2aT_sb=w2aT_sb,
                         w2gTn_sb=w2gTn_sb, wihT_sb=wihT_sb, whhT_sb=whhT_sb,
                         eye=eye, b1a=b1a, b1g=b1g, br_half=br_half,
                         bz_half=bz_half, bih_n=bih_n, bhh_n=bhh_n,
                         b2eff=b2eff, ones64div=ones64div, ones_row=ones_row),
                )

    _split_multi_waits(nc)
    return nc


def _emit_step(nc, tc, cpool, apool, bigpool, wpool, ppool, dpool,
               h_ap, newh_ap, ar_param, do_debate, C):
    w1aT_sb = C["w1aT_sb"]; w1gT_sb = C["w1gT_sb"]; w2aT_sb = C["w2aT_sb"]
    w2gTn_sb = C["w2gTn_sb"]; wihT_sb = C["wihT_sb"]; whhT_sb = C["whhT_sb"]
    eye = C["eye"]; b1a = C["b1a"]; b1g = C["b1g"]; br_half = C["br_half"]
    bz_half = C["bz_half"]; bih_n = C["bih_n"]; bhh_n = C["bhh_n"]
    b2eff = C["b2eff"]; ones64div = C["ones64div"]; ones_row = C["ones_row"]
    if True:
        if True:
            # ---- accumulators (one column per chunk) ----
            fmacc = apool.tile([128, N_CHUNKS], F32, tag="fmacc")
            sqacc = apool.tile([64, N_CHUNKS], F32, tag="sqacc")
            weoacc = apool.tile([64, N_CHUNKS], F32, tag="weoacc")
            eacc = apool.tile([1, N_CHUNKS], F32, tag="eacc")

            # feature-major pre-sync hidden state for the whole shard (8 MB)
            nh_fm = bigpool.tile([128, N_LOCAL], F32, tag="nh")

            # ---- pass 1 ----
            for c in range(N_CHUNKS):
                cs = c * CHUNK

                h_cm = wpool.tile([128, CHUNK], F32, tag="h_cm")
                nc.sync.dma_start(_r(h_cm[:]), _r(h_ap[c]))

                hT_ps = ppool.tile([128, CHUNK], F32, tag="hT", bufs=1)
                for g in range(GROUPS):
                    nc.tensor.transpose(
                        _r(hT_ps[:, g * 128 : (g + 1) * 128]),
                        _r(h_cm[:, g * 128 : (g + 1) * 128]),
                        _r(eye[:]),
                    )
                h_t = wpool.tile([128, CHUNK], F32, tag="h_t")
                nc.vector.tensor_copy(_r(h_t[:]), hT_ps[:])

                # layer 1 (a and g engines)
                z1a_ps = ppool.tile([128, CHUNK], F32, tag="z1ab", bufs=2)
                z1g_ps = ppool.tile([128, CHUNK], F32, tag="z1ab", bufs=2)
                nc.tensor.matmul(z1a_ps[:], _r(w1aT_sb[:]), _r(h_t[:]), start=True, stop=True)
                nc.tensor.matmul(z1g_ps[:], _r(w1gT_sb[:]), _r(h_t[:]), start=True, stop=True)
                a1 = wpool.tile([128, CHUNK], F32, tag="a1")
                g1 = wpool.tile([128, CHUNK], F32, tag="g1")
                nc.vector.tensor_scalar(_r(a1[:]), z1a_ps[:], b1a, 0.0, ALU.add, ALU.max)
                nc.vector.tensor_scalar(_r(g1[:]), z1g_ps[:], b1g, 0.0, ALU.add, ALU.max)

                # layer 2 fused: out_raw = W2a@a1 - W2g@g1   (bias folded out)
                outt_ps = ppool.tile([65, CHUNK], F32, tag="outt", bufs=1)
                nc.tensor.matmul(
                    outt_ps[0:OUT_D, :], _r(w2aT_sb[:]), _r(a1[:]), start=True, stop=False
                )
                nc.tensor.matmul(
                    outt_ps[0:OUT_D, :], _r(w2gTn_sb[:]), _r(g1[:]), start=False, stop=True
                )

                # t row: ones(1/64) @ (out+b2)^2
                sq = wpool.tile([64, CHUNK], F32, tag="sq")
                nc.scalar.activation(
                    _r(sq[:]),
                    outt_ps[0:OUT_D, :],
                    AF.Square,
                    bias=b2eff,
                    accum_out=sqacc[:, c : c + 1],
                )
                nc.tensor.matmul(
                    outt_ps[64:65, :],
                    ones64div[:],
                    sq[:],
                    start=True,
                    stop=True,
                    tile_position=(0, 64),
                )

                # gru input tile: rows 0:64 = out_raw, row 64 = t
                gru_in = wpool.tile([65, CHUNK], F32, tag="gru_in")
                nc.scalar.activation(_r(gru_in[:]), outt_ps[:], AF.Copy)

                # softmax pieces: e = exp(t); weo += out_raw * e
                e_row = wpool.tile([1, CHUNK], F32, tag="e_row")
                nc.scalar.activation(
                    _r(e_row[:]),
                    outt_ps[64:65, :],
                    AF.Exp,
                    accum_out=eacc[:, c : c + 1],
                )
                erep_ps = ppool.tile([64, CHUNK], F32, tag="tpb", bufs=1)
                nc.tensor.matmul(erep_ps[:], _r(ones_row[:]), _r(e_row[:]), start=True, stop=True)
                weo_scr = wpool.tile([64, CHUNK], F32, tag="weo")
                nc.vector.scalar_tensor_tensor(
                    weo_scr[:],
                    gru_in[0:OUT_D, :],
                    0.0,
                    erep_ps[:],
                    ALU.bypass,
                    ALU.mult,
                    accum_out=weoacc[:, c : c + 1],
                )

                # GRU gate matmuls (r/z accumulate gi+gh in PSUM)
                r_ps = ppool.tile([128, CHUNK], F32, tag="gates", bufs=3)
                z_ps = ppool.tile([128, CHUNK], F32, tag="gates", bufs=3)
                in_ps = ppool.tile([128, CHUNK], F32, tag="gates", bufs=3)
                hn_ps = ppool.tile([128, CHUNK], F32, tag="gates", bufs=3)
                nc.tensor.matmul(r_ps[:], _r(wihT_sb[:, 0:128]), _r(gru_in[:]), start=True, stop=False)
                nc.tensor.matmul(r_ps[:], _r(whhT_sb[:, 0:128]), _r(h_t[:]), start=False, stop=True)
                nc.tensor.matmul(z_ps[:], _r(wihT_sb[:, 128:256]), _r(gru_in[:]), start=True, stop=False)
                nc.tensor.matmul(z_ps[:], _r(whhT_sb[:, 128:256]), _r(h_t[:]), start=False, stop=True)
                nc.tensor.matmul(in_ps[:], _r(wihT_sb[:, 256:384]), _r(gru_in[:]), start=True, stop=True)
                nc.tensor.matmul(hn_ps[:], _r(whhT_sb[:, 256:384]), _r(h_t[:]), start=True, stop=True)

                # sigmoid via tanh: sig(v) = 0.5 + 0.5*tanh(v/2)
                r_sb = wpool.tile([128, CHUNK], F32, tag="r_sb")
                z_sb = wpool.tile([128, CHUNK], F32, tag="z_sb")
                nc.scalar.activation(r_sb[:], r_ps[:], AF.Tanh, bias=br_half, scale=0.5)
                nc.scalar.activation(z_sb[:], z_ps[:], AF.Tanh, bias=bz_half, scale=0.5)
                nc.gpsimd.tensor_scalar(r_sb[:], r_sb[:], 0.5, 0.5, ALU.mult, ALU.add)
                nc.gpsimd.tensor_scalar(z_sb[:], z_sb[:], 0.5, 0.5, ALU.mult, ALU.add)

                # n = tanh(i_n + bih_n + r*(h_n + bhh_n))
                rhn = wpool.tile([128, CHUNK], F32, tag="rhn")
                nc.vector.scalar_tensor_tensor(
                    rhn[:], hn_ps[:], bhh_n, r_sb[:], ALU.add, ALU.mult
                )
                nin = wpool.tile([128, CHUNK], F32, tag="nin")
                nc.vector.scalar_tensor_tensor(
                    nin[:], in_ps[:], bih_n, rhn[:], ALU.add, ALU.add
                )
                n_sb = wpool.tile([128, CHUNK], F32, tag="n_sb")
                nc.scalar.activation(n_sb[:], nin[:], AF.Tanh)

                # new_h = n + z*(h - n); accumulate faction-mean partials
                d_sb = wpool.tile([128, CHUNK], F32, tag="d_sb")
                nc.gpsimd.tensor_tensor(d_sb[:], h_t[:], n_sb[:], ALU.subtract)
                zd_sb = wpool.tile([128, CHUNK], F32, tag="zd_sb")
                nc.gpsimd.tensor_tensor(zd_sb[:], z_sb[:], d_sb[:], ALU.mult)
                nc.gpsimd.scalar_tensor_tensor(
                    _r(nh_fm[:, cs : cs + CHUNK]),
                    zd_sb[:],
                    0.0,
                    n_sb[:],
                    ALU.bypass,
                    ALU.add,
                    accum_out=fmacc[:, c : c + 1],
                )

            # ---- reductions + AllReduce ----
            fm_col = apool.tile([128, 1], F32, tag="fm_col")
            nc.vector.tensor_reduce(fm_col[:], fmacc[:], AX.X, ALU.add)
            fm_mean = apool.tile([128, 1], F32, tag="fm_mean")
            nc.vector.tensor_scalar(fm_mean[:], fm_col[:], 1.0 / N_LOCAL, None, ALU.mult)
            fm015 = apool.tile([128, 1], F32, tag="fm015")
            nc.vector.tensor_scalar(fm015[:], fm_col[:], SYNC / N_LOCAL, None, ALU.mult)

            sq_red = apool.tile([64, 1], F32, tag="sq_red")
            nc.vector.tensor_reduce(sq_red[:], sqacc[:], AX.X, ALU.add)
            weo_red = apool.tile([64, 1], F32, tag="weo_red")
            nc.vector.tensor_reduce(weo_red[:], weoacc[:], AX.X, ALU.add)
            e_red = apool.tile([1, 1], F32, tag="e_red")
            nc.vector.tensor_reduce(e_red[:], eacc[:], AX.X, ALU.add)

            # fm row [1,128], weo row [1,64], sum_t [1,1] — separate PSUM tiles,
            # all at partition 0 (matmul base-partition constraint)
            fmrow_ps = ppool.tile([1, 128], F32, tag="hT", bufs=1)
            weorow_ps = ppool.tile([1, 64], F32, tag="tpb", bufs=1)
            sqtot_ps = ppool.tile([1, 1], F32, tag="gates", bufs=3)
            nc.tensor.transpose(fmrow_ps[:], fm_mean[:], eye[:])
            nc.tensor.transpose(weorow_ps[:], weo_red[:], eye[0:64, 0:64])
            nc.tensor.matmul(
                sqtot_ps[:], sq_red[:], ones64div[:], start=True, stop=True
            )
            ar_sb = apool.tile([1, 256], F32, tag="ar_sb")
            nc.vector.memset(ar_sb[:], 0.0)
            nc.scalar.activation(ar_sb[0:1, 0:128], fmrow_ps[:], AF.Copy)
            nc.scalar.activation(ar_sb[0:1, 128:192], weorow_ps[:], AF.Copy)
            nc.scalar.activation(ar_sb[0:1, 192:193], sqtot_ps[:], AF.Copy)
            nc.vector.tensor_copy(ar_sb[0:1, 193:194], e_red[:])

            ar_in = dpool.tile([1, 256], F32, tag="ar_in")
            ar_out = dpool.tile([1, 256], F32, tag="ar_out")
            nc.sync.dma_start(ar_in[:], ar_sb[:])
            nc.gpsimd.collective_compute(
                "AllReduce",
                ALU.add,
                ins=[ar_in[:].opt()],
                outs=[ar_out[:].opt()],
                replica_groups=[list(range(N_CORES))],
            )
            nc.sync.dma_start(ar_param.ap(), ar_out[:].rearrange("a b -> (a b)"))

            go015 = apool.tile([128, 1], F32, tag="go015")
            if do_debate:
                arr_sb = apool.tile([1, 128], F32, tag="arr_sb")
                nc.sync.dma_start(arr_sb[:], ar_out[0:1, 0:128])
                go_ps = ppool.tile([128, 1], F32, tag="tpb", bufs=1)
                nc.tensor.transpose(go_ps[:], arr_sb[:], eye[0:1, 0:1])
                nc.scalar.activation(
                    go015[:], go_ps[:], AF.Copy, scale=DEBATE / N_CORES
                )

            # ---- pass 2: faction sync + debate + transpose + store ----
            order = list(range(DC_CHUNKS, N_CHUNKS)) + list(range(DC_CHUNKS))
            for c in order:
                cs = c * CHUNK
                s = nh_fm[:, cs : cs + CHUNK]
                nc.vector.tensor_scalar(_r(s), s, 1.0 - SYNC, fm015, ALU.mult, ALU.add)
                if do_debate and c < DC_CHUNKS:
                    nc.vector.tensor_scalar(
                        _r(s), s, 1.0 - DEBATE, go015, ALU.mult, ALU.add
                    )
                oT_ps = ppool.tile([128, CHUNK], F32, tag="z1ab", bufs=2)
                for g in range(GROUPS):
                    nc.tensor.transpose(
                        _r(oT_ps[:, g * 128 : (g + 1) * 128]),
                        _r(s[:, g * 128 : (g + 1) * 128]),
                        _r(eye[:]),
                    )
                o_sb = wpool.tile([128, CHUNK], F32, tag="o_sb")
                nc.scalar.activation(o_sb[:], oT_ps[:], AF.Copy)
                nc.sync.dma_start(newh_ap[c], o_sb[:])

_SPLIT_ENGINES = {
    mybir.EngineType.PE,
    mybir.EngineType.Activation,
    mybir.EngineType.DVE,
    mybir.EngineType.Pool,
    mybir.EngineType.SP,
}


def _split_multi_waits(nc, max_waits: int = 1):
    """Walrus (neuronxcc codegen) accepts at most one sync wait per engine
    instruction on trn2; the native bacc path splits extra waits into
    separate instructions (Bacc.generate_event_semaphores) but the bass2jax
    path does not. Hoist extra waits onto NoOps in front of the instruction."""
    for bb in nc.main_func.blocks:
        out = []
        for ins in bb.instructions:
            si = getattr(ins, "sync_info", None)
            if (
                si is not None
                and si.on_wait
                and len(si.on_wait) > max_waits
                and not isinstance(ins, mybir.InstEventSemaphore)
                and getattr(ins, "engine", None) in _SPLIT_ENGINES
            ):
                extra, keep = si.on_wait[:-max_waits], si.on_wait[-max_waits:]
                for w in extra:
                    nop = mybir.InstNoOp(
                        name=nc.get_next_instruction_name(),
                        engine=ins.engine,
                        ins=[],
                        outs=[],
                        sync_info=mybir.SyncInfo(on_wait=[w], on_update=[]),
                    )
                    out.append(nop)
                si.on_wait = keep
            out.append(ins)
        bb.instructions[:] = out


def _prep_host(inputs):
    x = inputs["x"].astype(np.float32)
    W1a, b1a = inputs["W1a"], inputs["b1a"]
    W1g, b1g = inputs["W1g"], inputs["b1g"]
    W2a, b2a = inputs["W2a"], inputs["b2a"]
    W2g, b2g = inputs["W2g"], inputs["b2g"]
    Wih, Whh = inputs["Wih"], inputs["Whh"]
    bih, bhh = inputs["bih"], inputs["bhh"]

    x0 = x[0]
    b1a_eff = b1a + W1a[:, :IN_D] @ x0
    b1g_eff = b1g + W1g[:, :IN_D] @ x0
    b2eff = b2a - b2g
    bih_eff = bih + Wih[:, :OUT_D] @ b2eff
    bias_rz = bih_eff[: 2 * HID_D] + bhh[: 2 * HID_D]

    biases = np.zeros((128, 8), dtype=np.float32)
    biases[:, 0] = b1a_eff
    biases[:, 1] = b1g_eff
    biases[:, 2] = 0.5 * bias_rz[:HID_D]
    biases[:, 3] = 0.5 * bias_rz[HID_D : 2 * HID_D]
    biases[:, 4] = bih_eff[2 * HID_D :]
    biases[:, 5] = bhh[2 * HID_D :]
    biases[:OUT_D, 6] = b2eff

    common = {
        "w1aT": np.ascontiguousarray(W1a[:, IN_D:].T),
        "w1gT": np.ascontiguousarray(W1g[:, IN_D:].T),
        "w2aT": np.ascontiguousarray(W2a.T),
        "w2gTn": np.ascontiguousarray(-W2g.T),
        "wihT": np.ascontiguousarray(Wih.T),
        "whhT": np.ascontiguousarray(Whh.T),
        "eye": np.eye(128, dtype=np.float32),
        "ones_col": np.full((64, 1), 1.0 / 64.0, dtype=np.float32),
        "ones_row": np.ones((1, 64), dtype=np.float32),
        "biases": biases,
    }
    common = {k: v.astype(np.float32) for k, v in common.items()}
    return common, b2eff.astype(np.float32)


def kernel(**inputs):
    step = int(np.asarray(inputs["step"]))
    do_debate = step > 5

    key = ("nc", do_debate)
    if key not in _CACHED:
        _CACHED[key] = build_nc(do_debate)
    nc = _CACHED[key]

    common, b2eff = _prep_host(inputs)
    h = np.ascontiguousarray(inputs["hiddens"].astype(np.float32))
    in_maps = []
    for c in range(N_CORES):
        m = dict(common)
        m["h"] = np.ascontiguousarray(h[c * N_LOCAL : (c + 1) * N_LOCAL])
        in_maps.append(m)

    import os

    trace = bool(os.environ.get("KERNEL_TRACE"))
    kw = {}
    if trace:
        kw = {"trace": True, "tmpdir": os.environ.get("KERNEL_TRACE_DIR") or None}
    res = run_bass_kernel_spmd(nc, in_maps, list(range(N_CORES)), **kw)
    if res.exec_time_ns is not None:
        print(f"HW exec time: {res.exec_time_ns} ns")
        if res.mean_exec_time_ns is not None:
            print(f"HW exec time mean: {res.mean_exec_time_ns:.0f} ns (max core {res.max_exec_time_core_id})")
    outs = res.results

    new_h = np.concatenate([outs[c]["new_h"] for c in range(N_CORES)], axis=0)
    ar = outs[0]["ar_out"]
    weo = ar[128 : 128 + OUT_D]
    sum_t = ar[192]
    sum_e = ar[193]
    combined = weo / sum_e + b2eff
    pred = (combined @ inputs["Wo"].T + inputs["bo"])[None, :].astype(np.float32)
    t_mean = np.float32(sum_t / N_CELLS)
    return pred, t_mean, new_h


if __name__ == "__main__":
    import reference as R

    inp = R.setup_inputs()
    inp = {k: np.asarray(v) for k, v in inp.items()}
    pred, tm, nh = kernel(**inp)
    print("pred", pred[0, :4], "t_mean", tm, "new_h", nh.shape, nh[0, :4])


# revision 17
# speedup vs baseline: 358.6643x; 358.6643x over previous
"""Trainium2 Bass kernel for nn_BaseEngine (8-core SPMD).

Computation (per reference):
  c = [x_bcast, h]  [N,192]
  out = (relu(c@W1a.T+b1a)@W2a.T+b2a) - (relu(c@W1g.T+b1g)@W2g.T+b2g)
  t = mean(out^2, -1)
  new_h = GRU([out, t], h)
  new_h = faction_sync(new_h, 8 factions, step)
  pred = softmax(t) @ out @ Wo.T + bo

Sharding: n_cells across 8 cores (16384 each); each faction == one core's
shard so faction means are core-local. One 1KB AllReduce carries
[faction_mean_row, weighted_out_row, sum_t, sum_exp_t].

Device layout: feature-major ([feature, cell] on [partition, free]) with
PE transposes at load/store. x is folded into layer-1 biases on the host,
b2 into the GRU input bias. Sigmoid is computed as 0.5+0.5*tanh(v/2) so
every ACT function stays in the one "exp_and_others" table set.
"""

import sys

sys.path.insert(0, "/opt/trn_rl_repo")

import numpy as np

import concourse.bass as bass
import concourse.mybir as mybir
from concourse import tile
from concourse.bass_utils import run_bass_kernel_spmd

N_CELLS = 131072
N_CORES = 8
N_LOCAL = N_CELLS // N_CORES  # 16384
CHUNK = 512
N_CHUNKS = N_LOCAL // CHUNK  # 32
GROUPS = CHUNK // 128  # 4
IN_D, HID_D, OUT_D = 64, 128, 64
SYNC, DEBATE = 0.15, 0.15
DC_CHUNKS = (N_LOCAL // 4) // CHUNK  # debate applies to first fs//4 cells

F32 = mybir.dt.float32
F32R = mybir.dt.float32r


def _r(ap):
    return ap.bitcast(F32R)
AF = mybir.ActivationFunctionType
ALU = mybir.AluOpType
AX = mybir.AxisListType

_CACHED = {}


def build_nc(do_debate: bool, repeat: int = 1) -> bass.Bass:
    nc = bass.Bass()

    h_in = nc.declare_dram_parameter("h", [N_LOCAL, HID_D], F32, isOutput=False)
    w1aT = nc.declare_dram_parameter("w1aT", [HID_D, 128], F32, isOutput=False)
    w1gT = nc.declare_dram_parameter("w1gT", [HID_D, 128], F32, isOutput=False)
    w2aT = nc.declare_dram_parameter("w2aT", [128, OUT_D], F32, isOutput=False)
    w2gTn = nc.declare_dram_parameter("w2gTn", [128, OUT_D], F32, isOutput=False)
    wihT = nc.declare_dram_parameter("wihT", [OUT_D + 1, 3 * HID_D], F32, isOutput=False)
    whhT = nc.declare_dram_parameter("whhT", [HID_D, 3 * HID_D], F32, isOutput=False)
    eye_in = nc.declare_dram_parameter("eye", [128, 128], F32, isOutput=False)
    ones_col_in = nc.declare_dram_parameter("ones_col", [64, 1], F32, isOutput=False)
    ones_row_in = nc.declare_dram_parameter("ones_row", [1, 64], F32, isOutput=False)
    bias_in = nc.declare_dram_parameter("biases", [128, 8], F32, isOutput=False)

    newh_out = nc.declare_dram_parameter("new_h", [N_LOCAL, HID_D], F32, isOutput=True)
    ar_param = nc.declare_dram_parameter("ar_out", [256], F32, isOutput=True)

    # [chunk, partition, (group, feat)] views of the cell-major DRAM tensors
    h_ap = h_in.ap().rearrange("(c g p) f -> c p g f", g=GROUPS, p=128)
    newh_ap = newh_out.ap().rearrange("(c g p) f -> c p g f", g=GROUPS, p=128)

    with tile.TileContext(nc) as tc:
        with (
            tc.tile_pool(name="const", bufs=1) as cpool,
            tc.tile_pool(name="acc", bufs=1) as apool,
            tc.tile_pool(name="big", bufs=1) as bigpool,
            tc.tile_pool(name="work", bufs=3) as wpool,
            tc.tile_pool(name="psum", bufs=2, space="PSUM") as ppool,
            tc.tile_pool(name="dram", bufs=1, space="DRAM") as dpool,
        ):
            # ---- constants / weights ----
            w1aT_sb = cpool.tile([HID_D, 128], F32, tag="w1a")
            w1gT_sb = cpool.tile([HID_D, 128], F32, tag="w1g")
            w2aT_sb = cpool.tile([128, OUT_D], F32, tag="w2a")
            w2gTn_sb = cpool.tile([128, OUT_D], F32, tag="w2g")
            wihT_sb = cpool.tile([OUT_D + 1, 3 * HID_D], F32, tag="wih")
            whhT_sb = cpool.tile([HID_D, 3 * HID_D], F32, tag="whh")
            eye = cpool.tile([128, 128], F32, tag="eye")
            bias_sb = cpool.tile([128, 8], F32, tag="bias")
            ones64div = cpool.tile([64, 1], F32, tag="o64")
            ones_row = cpool.tile([1, 64], F32, tag="orow")
            nc.sync.dma_start(_r(ones64div[:]), _r(ones_col_in.ap()))
            nc.sync.dma_start(_r(ones_row[:]), _r(ones_row_in.ap()))
            nc.sync.dma_start(_r(w1aT_sb[:]), _r(w1aT.ap()))
            nc.sync.dma_start(_r(w1gT_sb[:]), _r(w1gT.ap()))
            nc.sync.dma_start(_r(w2aT_sb[:]), _r(w2aT.ap()))
            nc.sync.dma_start(_r(w2gTn_sb[:]), _r(w2gTn.ap()))
            nc.sync.dma_start(_r(wihT_sb[:]), _r(wihT.ap()))
            nc.sync.dma_start(_r(whhT_sb[:]), _r(whhT.ap()))
            nc.sync.dma_start(_r(eye[:]), _r(eye_in.ap()))
            nc.sync.dma_start(_r(bias_sb[:]), _r(bias_in.ap()))

            # Pre-touch every PE-consumed tile with a tiny K=1 matmul so the
            # PE vector clock observes the load semaphores up front; real
            # matmuls then carry at most one sync wait (walrus's LDWEIGHTS
            # struct can't encode more).
            touch_ps = ppool.tile([1, 16], F32, tag="hT", bufs=1)
            for i, w in enumerate(
                [w1aT_sb, w1gT_sb, w2aT_sb, w2gTn_sb, wihT_sb, whhT_sb,
                 eye, ones64div, ones_row]
            ):
                nc.tensor.matmul(
                    touch_ps[0:1, i : i + 1], w[0:1, 0:1], w[0:1, 0:1],
                    start=True, stop=True,
                )

            b1a = bias_sb[:, 0:1]
            b1g = bias_sb[:, 1:2]
            br_half = bias_sb[:, 2:3]
            bz_half = bias_sb[:, 3:4]
            bih_n = bias_sb[:, 4:5]
            bhh_n = bias_sb[:, 5:6]
            b2eff = bias_sb[0:OUT_D, 6:7]

            for _rep in range(repeat):
                _emit_step(
                    nc, tc, cpool, apool, bigpool, wpool, ppool, dpool,
                    h_ap if _rep == 0 else newh_ap, newh_ap, ar_param,
                    do_debate,
                    dict(w1aT_sb=w1aT_sb, w1gT_sb=w1gT_sb, w2aT_sb=w2aT_sb,
                         w2gTn_sb=w2gTn_sb, wihT_sb=wihT_sb, whhT_sb=whhT_sb,
                         eye=eye, b1a=b1a, b1g=b1g, br_half=br_half,
                         bz_half=bz_half, bih_n=bih_n, bhh_n=bhh_n,
                         b2eff=b2eff, ones64div=ones64div, ones_row=ones_row),
                )

    _split_multi_waits(nc)
    return nc


def _emit_step(nc, tc, cpool, apool, bigpool, wpool, ppool, dpool,
               h_ap, newh_ap, ar_param, do_debate, C):
    w1aT_sb = C["w1aT_sb"]; w1gT_sb = C["w1gT_sb"]; w2aT_sb = C["w2aT_sb"]
    w2gTn_sb = C["w2gTn_sb"]; wihT_sb = C["wihT_sb"]; whhT_sb = C["whhT_sb"]
    eye = C["eye"]; b1a = C["b1a"]; b1g = C["b1g"]; br_half = C["br_half"]
    bz_half = C["bz_half"]; bih_n = C["bih_n"]; bhh_n = C["bhh_n"]
    b2eff = C["b2eff"]; ones64div = C["ones64div"]; ones_row = C["ones_row"]
    if True:
        if True:
            # ---- accumulators (one column per chunk) ----
            fmacc = apool.tile([128, N_CHUNKS], F32, tag="fmacc")
            sqacc = apool.tile([64, N_CHUNKS], F32, tag="sqacc")
            weoacc = apool.tile([64, N_CHUNKS], F32, tag="weoacc")
            eacc = apool.tile([1, N_CHUNKS], F32, tag="eacc")

            # feature-major pre-sync hidden state for the whole shard (8 MB)
            nh_fm = bigpool.tile([128, N_LOCAL], F32, tag="nh")

            # ---- pass 1 ----
            for c in range(N_CHUNKS):
                cs = c * CHUNK

                h_cm = wpool.tile([128, CHUNK], F32, tag="h_cm")
                nc.sync.dma_start(_r(h_cm[:]), _r(h_ap[c]))

                hT_ps = ppool.tile([128, CHUNK], F32, tag="hT", bufs=1)
                for g in range(GROUPS):
                    nc.tensor.transpose(
                        _r(hT_ps[:, g * 128 : (g + 1) * 128]),
                        _r(h_cm[:, g * 128 : (g + 1) * 128]),
                        _r(eye[:]),
                    )
                h_t = wpool.tile([128, CHUNK], F32, tag="h_t")
                nc.vector.tensor_copy(_r(h_t[:]), hT_ps[:])

                # layer 1 (a and g engines)
                z1a_ps = ppool.tile([128, CHUNK], F32, tag="z1ab", bufs=2)
                z1g_ps = ppool.tile([128, CHUNK], F32, tag="z1ab", bufs=2)
                nc.tensor.matmul(z1a_ps[:], _r(w1aT_sb[:]), _r(h_t[:]), start=True, stop=True)
                nc.tensor.matmul(z1g_ps[:], _r(w1gT_sb[:]), _r(h_t[:]), start=True, stop=True)
                a1 = wpool.tile([128, CHUNK], F32, tag="a1")
                g1 = wpool.tile([128, CHUNK], F32, tag="g1")
                nc.vector.tensor_scalar(_r(a1[:]), z1a_ps[:], b1a, 0.0, ALU.add, ALU.max)
                nc.vector.tensor_scalar(_r(g1[:]), z1g_ps[:], b1g, 0.0, ALU.add, ALU.max)

                # layer 2 fused: out_raw = W2a@a1 - W2g@g1   (bias folded out)
                outt_ps = ppool.tile([65, CHUNK], F32, tag="outt", bufs=1)
                nc.tensor.matmul(
                    outt_ps[0:OUT_D, :], _r(w2aT_sb[:]), _r(a1[:]), start=True, stop=False
                )
                nc.tensor.matmul(
                    outt_ps[0:OUT_D, :], _r(w2gTn_sb[:]), _r(g1[:]), start=False, stop=True
                )

                # t row: ones(1/64) @ (out+b2)^2
                sq = wpool.tile([64, CHUNK], F32, tag="sq")
                nc.scalar.activation(
                    _r(sq[:]),
                    outt_ps[0:OUT_D, :],
                    AF.Square,
                    bias=b2eff,
                    accum_out=sqacc[:, c : c + 1],
                )
                nc.tensor.matmul(
                    outt_ps[64:65, :],
                    ones64div[:],
                    sq[:],
                    start=True,
                    stop=True,
                    tile_position=(0, 64),
                )

                # gru input tile: rows 0:64 = out_raw, row 64 = t
                gru_in = wpool.tile([65, CHUNK], F32, tag="gru_in")
                nc.scalar.activation(_r(gru_in[:]), outt_ps[:], AF.Copy)

                # softmax pieces: e = exp(t); weo += out_raw * e
                e_row = wpool.tile([1, CHUNK], F32, tag="e_row")
                nc.scalar.activation(
                    _r(e_row[:]),
                    outt_ps[64:65, :],
                    AF.Exp,
                    accum_out=eacc[:, c : c + 1],
                )
                erep_ps = ppool.tile([64, CHUNK], F32, tag="tpb", bufs=1)
                nc.tensor.matmul(erep_ps[:], _r(ones_row[:]), _r(e_row[:]), start=True, stop=True)
                weo_scr = wpool.tile([64, CHUNK], F32, tag="weo")
                nc.vector.scalar_tensor_tensor(
                    weo_scr[:],
                    gru_in[0:OUT_D, :],
                    0.0,
                    erep_ps[:],
                    ALU.bypass,
                    ALU.mult,
                    accum_out=weoacc[:, c : c + 1],
                )

                # GRU gate matmuls (r/z accumulate gi+gh in PSUM)
                r_ps = ppool.tile([128, CHUNK], F32, tag="gates", bufs=3)
                z_ps = ppool.tile([128, CHUNK], F32, tag="gates", bufs=3)
                in_ps = ppool.tile([128, CHUNK], F32, tag="gates", bufs=3)
                hn_ps = ppool.tile([128, CHUNK], F32, tag="gates", bufs=3)
                nc.tensor.matmul(r_ps[:], _r(wihT_sb[:, 0:128]), _r(gru_in[:]), start=True, stop=False)
                nc.tensor.matmul(r_ps[:], _r(whhT_sb[:, 0:128]), _r(h_t[:]), start=False, stop=True)
                nc.tensor.matmul(z_ps[:], _r(wihT_sb[:, 128:256]), _r(gru_in[:]), start=True, stop=False)
                nc.tensor.matmul(z_ps[:], _r(whhT_sb[:, 128:256]), _r(h_t[:]), start=False, stop=True)
                nc.tensor.matmul(in_ps[:], _r(wihT_sb[:, 256:384]), _r(gru_in[:]), start=True, stop=True)
                nc.tensor.matmul(hn_ps[:], _r(whhT_sb[:, 256:384]), _r(h_t[:]), start=True, stop=True)

                # sigmoid via tanh: sig(v) = 0.5 + 0.5*tanh(v/2)
                r_sb = wpool.tile([128, CHUNK], F32, tag="r_sb")
                z_sb = wpool.tile([128, CHUNK], F32, tag="z_sb")
                nc.scalar.activation(r_sb[:], r_ps[:], AF.Tanh, bias=br_half, scale=0.5)
                nc.scalar.activation(z_sb[:], z_ps[:], AF.Tanh, bias=bz_half, scale=0.5)
                nc.gpsimd.tensor_scalar(r_sb[:], r_sb[:], 0.5, 0.5, ALU.mult, ALU.add)
                nc.gpsimd.tensor_scalar(z_sb[:], z_sb[:], 0.5, 0.5, ALU.mult, ALU.add)

                # n = tanh(i_n + bih_n + r*(h_n + bhh_n))
                rhn = wpool.tile([128, CHUNK], F32, tag="rhn")
                nc.vector.scalar_tensor_tensor(
                    rhn[:], hn_ps[:], bhh_n, r_sb[:], ALU.add, ALU.mult
                )
                nin = wpool.tile([128, CHUNK], F32, tag="nin")
                nc.vector.scalar_tensor_tensor(
                    nin[:], in_ps[:], bih_n, rhn[:], ALU.add, ALU.add
                )
                n_sb = wpool.tile([128, CHUNK], F32, tag="n_sb")
                nc.scalar.activation(n_sb[:], nin[:], AF.Tanh)

                # new_h = n + z*(h - n); accumulate faction-mean partials
                d_sb = wpool.tile([128, CHUNK], F32, tag="d_sb")
                nc.gpsimd.tensor_tensor(d_sb[:], h_t[:], n_sb[:], ALU.subtract)
                zd_sb = wpool.tile([128, CHUNK], F32, tag="zd_sb")
                nc.gpsimd.tensor_tensor(zd_sb[:], z_sb[:], d_sb[:], ALU.mult)
                nc.vector.scalar_tensor_tensor(
                    _r(nh_fm[:, cs : cs + CHUNK]),
                    zd_sb[:],
                    0.0,
                    n_sb[:],
                    ALU.bypass,
                    ALU.add,
                    accum_out=fmacc[:, c : c + 1],
                )

            # ---- reductions + AllReduce ----
            fm_col = apool.tile([128, 1], F32, tag="fm_col")
            nc.vector.tensor_reduce(fm_col[:], fmacc[:], AX.X, ALU.add)
            fm_mean = apool.tile([128, 1], F32, tag="fm_mean")
            nc.vector.tensor_scalar(fm_mean[:], fm_col[:], 1.0 / N_LOCAL, None, ALU.mult)
            fm015 = apool.tile([128, 1], F32, tag="fm015")
            nc.vector.tensor_scalar(fm015[:], fm_col[:], SYNC / N_LOCAL, None, ALU.mult)

            sq_red = apool.tile([64, 1], F32, tag="sq_red")
            nc.vector.tensor_reduce(sq_red[:], sqacc[:], AX.X, ALU.add)
            weo_red = apool.tile([64, 1], F32, tag="weo_red")
            nc.vector.tensor_reduce(weo_red[:], weoacc[:], AX.X, ALU.add)
            e_red = apool.tile([1, 1], F32, tag="e_red")
            nc.vector.tensor_reduce(e_red[:], eacc[:], AX.X, ALU.add)

            # fm row [1,128], weo row [1,64], sum_t [1,1] — separate PSUM tiles,
            # all at partition 0 (matmul base-partition constraint)
            fmrow_ps = ppool.tile([1, 128], F32, tag="hT", bufs=1)
            weorow_ps = ppool.tile([1, 64], F32, tag="tpb", bufs=1)
            sqtot_ps = ppool.tile([1, 1], F32, tag="gates", bufs=3)
            nc.tensor.transpose(fmrow_ps[:], fm_mean[:], eye[:])
            nc.tensor.transpose(weorow_ps[:], weo_red[:], eye[0:64, 0:64])
            nc.tensor.matmul(
                sqtot_ps[:], sq_red[:], ones64div[:], start=True, stop=True
            )
            ar_sb = apool.tile([1, 256], F32, tag="ar_sb")
            nc.vector.memset(ar_sb[:], 0.0)
            nc.scalar.activation(ar_sb[0:1, 0:128], fmrow_ps[:], AF.Copy)
            nc.scalar.activation(ar_sb[0:1, 128:192], weorow_ps[:], AF.Copy)
            nc.scalar.activation(ar_sb[0:1, 192:193], sqtot_ps[:], AF.Copy)
            nc.vector.tensor_copy(ar_sb[0:1, 193:194], e_red[:])

            ar_in = dpool.tile([1, 256], F32, tag="ar_in")
            ar_out = dpool.tile([1, 256], F32, tag="ar_out")
            nc.sync.dma_start(ar_in[:], ar_sb[:])
            nc.gpsimd.collective_compute(
                "AllReduce",
                ALU.add,
                ins=[ar_in[:].opt()],
                outs=[ar_out[:].opt()],
                replica_groups=[list(range(N_CORES))],
            )
            nc.sync.dma_start(ar_param.ap(), ar_out[:].rearrange("a b -> (a b)"))

            go015 = apool.tile([128, 1], F32, tag="go015")
            if do_debate:
                arr_sb = apool.tile([1, 128], F32, tag="arr_sb")
                nc.sync.dma_start(arr_sb[:], ar_out[0:1, 0:128])
                go_ps = ppool.tile([128, 1], F32, tag="tpb", bufs=1)
                nc.tensor.transpose(go_ps[:], arr_sb[:], eye[0:1, 0:1])
                nc.scalar.activation(
                    go015[:], go_ps[:], AF.Copy, scale=DEBATE / N_CORES
                )

            # ---- pass 2: faction sync + debate + transpose + store ----
            order = list(range(DC_CHUNKS, N_CHUNKS)) + list(range(DC_CHUNKS))
            for c in order:
                cs = c * CHUNK
                s = nh_fm[:, cs : cs + CHUNK]
                nc.vector.tensor_scalar(_r(s), s, 1.0 - SYNC, fm015, ALU.mult, ALU.add)
                if do_debate and c < DC_CHUNKS:
                    nc.vector.tensor_scalar(
                        _r(s), s, 1.0 - DEBATE, go015, ALU.mult, ALU.add
                    )
                oT_ps = ppool.tile([128, CHUNK], F32, tag="z1ab", bufs=2)
                for g in range(GROUPS):
                    nc.tensor.transpose(
                        _r(oT_ps[:, g * 128 : (g + 1) * 128]),
                        _r(s[:, g * 128 : (g + 1) * 128]),
                        _r(eye[:]),
                    )
                o_sb = wpool.tile([128, CHUNK], F32, tag="o_sb")
                nc.scalar.activation(o_sb[:], oT_ps[:], AF.Copy)
                nc.sync.dma_start(newh_ap[c], o_sb[:])

_SPLIT_ENGINES = {
    mybir.EngineType.PE,
    mybir.EngineType.Activation,
    mybir.EngineType.DVE,
    mybir.EngineType.Pool,
    mybir.EngineType.SP,
}


def _split_multi_waits(nc, max_waits: int = 1):
    """Walrus (neuronxcc codegen) accepts at most one sync wait per engine
    instruction on trn2; the native bacc path splits extra waits into
    separate instructions (Bacc.generate_event_semaphores) but the bass2jax
    path does not. Hoist extra waits onto NoOps in front of the instruction."""
    for bb in nc.main_func.blocks:
        out = []
        for ins in bb.instructions:
            si = getattr(ins, "sync_info", None)
            if (
                si is not None
                and si.on_wait
                and len(si.on_wait) > max_waits
                and not isinstance(ins, mybir.InstEventSemaphore)
                and getattr(ins, "engine", None) in _SPLIT_ENGINES
            ):
                extra, keep = si.on_wait[:-max_waits], si.on_wait[-max_waits:]
                for w in extra:
                    nop = mybir.InstNoOp(
                        name=nc.get_next_instruction_name(),
                        engine=ins.engine,
                        ins=[],
                        outs=[],
                        sync_info=mybir.SyncInfo(on_wait=[w], on_update=[]),
                    )
                    out.append(nop)
                si.on_wait = keep
            out.append(ins)
        bb.instructions[:] = out


def _prep_host(inputs):
    x = inputs["x"].astype(np.float32)
    W1a, b1a = inputs["W1a"], inputs["b1a"]
    W1g, b1g = inputs["W1g"], inputs["b1g"]
    W2a, b2a = inputs["W2a"], inputs["b2a"]
    W2g, b2g = inputs["W2g"], inputs["b2g"]
    Wih, Whh = inputs["Wih"], inputs["Whh"]
    bih, bhh = inputs["bih"], inputs["bhh"]

    x0 = x[0]
    b1a_eff = b1a + W1a[:, :IN_D] @ x0
    b1g_eff = b1g + W1g[:, :IN_D] @ x0
    b2eff = b2a - b2g
    bih_eff = bih + Wih[:, :OUT_D] @ b2eff
    bias_rz = bih_eff[: 2 * HID_D] + bhh[: 2 * HID_D]

    biases = np.zeros((128, 8), dtype=np.float32)
    biases[:, 0] = b1a_eff
    biases[:, 1] = b1g_eff
    biases[:, 2] = 0.5 * bias_rz[:HID_D]
    biases[:, 3] = 0.5 * bias_rz[HID_D : 2 * HID_D]
    biases[:, 4] = bih_eff[2 * HID_D :]
    biases[:, 5] = bhh[2 * HID_D :]
    biases[:OUT_D, 6] = b2eff

    common = {
        "w1aT": np.ascontiguousarray(W1a[:, IN_D:].T),
        "w1gT": np.ascontiguousarray(W1g[:, IN_D:].T),
        "w2aT": np.ascontiguousarray(W2a.T),
        "w2gTn": np.ascontiguousarray(-W2g.T),
        "wihT": np.ascontiguousarray(Wih.T),
        "whhT": np.ascontiguousarray(Whh.T),
        "eye": np.eye(128, dtype=np.float32),
        "ones_col": np.full((64, 1), 1.0 / 64.0, dtype=np.float32),
        "ones_row": np.ones((1, 64), dtype=np.float32),
        "biases": biases,
    }
    common = {k: v.astype(np.float32) for k, v in common.items()}
    return common, b2eff.astype(np.float32)


def kernel(**inputs):
    step = int(np.asarray(inputs["step"]))
    do_debate = step > 5

    key = ("nc", do_debate)
    if key not in _CACHED:
        _CACHED[key] = build_nc(do_debate)
    nc = _CACHED[key]

    common, b2eff = _prep_host(inputs)
    h = np.ascontiguousarray(inputs["hiddens"].astype(np.float32))
    in_maps = []
    for c in range(N_CORES):
        m = dict(common)
        m["h"] = np.ascontiguousarray(h[c * N_LOCAL : (c + 1) * N_LOCAL])
        in_maps.append(m)

    import os

    trace = bool(os.environ.get("KERNEL_TRACE"))
    kw = {}
    if trace:
        kw = {"trace": True, "tmpdir": os.environ.get("KERNEL_TRACE_DIR") or None}
    res = run_bass_kernel_spmd(nc, in_maps, list(range(N_CORES)), **kw)
    if res.exec_time_ns is not None:
        print(f"HW exec time: {res.exec_time_ns} ns")
        if res.mean_exec_time_ns is not None:
            print(f"HW exec time mean: {res.mean_exec_time_ns:.0f} ns (max core {res.max_exec_time_core_id})")
    outs = res.results

    new_h = np.concatenate([outs[c]["new_h"] for c in range(N_CORES)], axis=0)
    ar = outs[0]["ar_out"]
    weo = ar[128 : 128 + OUT_D]
    sum_t = ar[192]
    sum_e = ar[193]
    combined = weo / sum_e + b2eff
    pred = (combined @ inputs["Wo"].T + inputs["bo"])[None, :].astype(np.float32)
    t_mean = np.float32(sum_t / N_CELLS)
    return pred, t_mean, new_h


if __name__ == "__main__":
    import reference as R

    inp = R.setup_inputs()
    inp = {k: np.asarray(v) for k, v in inp.items()}
    pred, tm, nh = kernel(**inp)
    print("pred", pred[0, :4], "t_mean", tm, "new_h", nh.shape, nh[0, :4])
